# revision 4
# baseline (speedup 1.0000x reference)
"""Causal self-attention Trainium2 Bass kernel (v4: continuous pipeline).

Problem: B=2, T=4096, C=512, H=8 heads, D=64.
  q = x@Wq.T, k = x@Wk.T, v = x@Wv.T  (per-head split)
  att = softmax(causal(q k^T / sqrt(D)));  y = att @ v;  out = y @ Wout.T

Sharding: 8 cores = 2 batches x 4 head-groups (2 heads each).
Each core computes, for its batch b and heads {2g, 2g+1}:
  - feature-major qT,kT [128, T] bf16 via PE matmuls (bf16, 1 cyc/row)
  - one continuous software pipeline over all (chunk c, t2-block j) pairs,
    crossing chunk boundaries without draining: transposed scores
    ST[t2, t1] = kT^T qT (two 64-partition row-groups run concurrently on
    the PE), exp on ACT (scale=1/8) over the causally-valid column range,
    diagonal 128x128 tiles masked multiplicatively (0/1 bf16, DVE) after
    exp, then yT_aug[65, t1] accumulation (ones column -> denominators).
  - QKV projections for the next chunk and the previous chunk's
    normalization/out-projection are interleaved as paced PE "fillers" so
    the ACT (exp) engine -- the bottleneck -- never starves; the
    reciprocal is split into 4 quarters to keep the in-order DVE queue
    responsive.
Host sums the 4 partial outputs per batch (row-parallel out projection).
"""

import os
import sys

import numpy as np

B, T, C = 2, 4096, 512
H, D = 8, 64
P = 128          # partitions / t2-block size
CH = 512         # t1 chunk width
NCH = T // CH    # 8 chunks
NTB = T // P     # 32 t-blocks
KC = C // P      # 4 contraction chunks for projections

_COMPILED = None


def _import_concourse():
    try:
        import concourse.bass  # noqa: F401
    except ImportError:
        for p in ("/opt/trn_rl_repo", os.path.expanduser("~/.axon_site/_ro/trn_rl_repo")):
            if os.path.isdir(p) and p not in sys.path:
                sys.path.insert(0, p)
        import concourse.bass  # noqa: F401


def _build():
    """Build + compile the SPMD Bass program (same program on all 8 cores)."""
    _import_concourse()
    import concourse.bass as bass  # noqa: F401
    import concourse.tile as tile
    from concourse import bacc, mybir

    f32 = mybir.dt.float32
    f32r = mybir.dt.float32r
    bf16 = mybir.dt.bfloat16
    EXP = mybir.ActivationFunctionType.Exp

    nc = bacc.Bacc("TRN2", target_bir_lowering=False, debug=False, num_devices=8)

    xT_d = nc.dram_tensor("xT", [C, T], bf16, kind="ExternalInput").ap()
    wq_d = nc.dram_tensor("wq", [P, C], bf16, kind="ExternalInput").ap()
    wk_d = nc.dram_tensor("wk", [P, C], bf16, kind="ExternalInput").ap()
    wv_d = nc.dram_tensor("wv", [P, C], bf16, kind="ExternalInput").ap()
    wo_d = nc.dram_tensor("wo", [P, C], bf16, kind="ExternalInput").ap()
    mk_d = nc.dram_tensor("mk", [P, P], bf16, kind="ExternalInput").ap()
    sel_d = nc.dram_tensor("sel", [65, P], f32r, kind="ExternalInput").ap()
    id_d = nc.dram_tensor("idm", [P, P], bf16, kind="ExternalInput").ap()
    out_d = nc.dram_tensor("out", [T, C], f32, kind="ExternalOutput").ap()

    import contextlib

    with tile.TileContext(nc) as tc, contextlib.ExitStack() as _pctx:
        # ---- persistent SBUF tensors
        persist = _pctx.enter_context(tc.tile_pool(name="persist", bufs=1))

        def ptile(shape, name, dt=f32):
            return persist.tile(shape, dt, name=name, tag=name)

        xT_sb = ptile([P, KC * T], "xT_sb", bf16)      # 4 MB
        wq_sb = ptile([P, C], "wq_sb", bf16)
        wk_sb = ptile([P, C], "wk_sb", bf16)
        wv_sb = ptile([P, C], "wv_sb", bf16)
        wo_sb = ptile([P, C], "wo_sb", bf16)
        mk_sb = ptile([P, P], "mk_sb", bf16)           # 0/1 lower-valid mask
        sel_sb = ptile([65, P], "sel_sb", f32r)
        id_sb = ptile([P, P], "id_sb", bf16)
        qT_sb = ptile([P, T], "qT_sb", bf16)
        kT_sb = ptile([P, T], "kT_sb", bf16)
        va1_sb = ptile([P, NTB * 65], "va1_sb", bf16)
        va2_sb = ptile([P, NTB * 65], "va2_sb", bf16)
        yT_sb = ptile([P, T], "yT_sb", bf16)

        # ---- input DMAs: q/k weights + first xT quarter first so chunk-0
        # QKV starts ASAP; the rest follows.
        TQ = T // 4

        def dma_xq(q):
            for k in range(KC):
                nc.sync.dma_start(
                    xT_sb[:, T * k + TQ * q : T * k + TQ * (q + 1)],
                    xT_d[P * k : P * (k + 1), TQ * q : TQ * (q + 1)],
                )

        nc.sync.dma_start(wq_sb[:], wq_d[:])
        nc.sync.dma_start(wk_sb[:], wk_d[:])
        dma_xq(0)
        nc.sync.dma_start(wv_sb[:], wv_d[:])
        nc.sync.dma_start(wo_sb[:], wo_d[:])
        nc.sync.dma_start(sel_sb[:], sel_d[:])
        nc.sync.dma_start(id_sb[:], id_d[:])
        nc.sync.dma_start(mk_sb[:], mk_d[:])
        for q in range(1, 4):
            dma_xq(q)
        ones_sb = ptile([P, NTB], "ones_sb", bf16)  # staging for vaug ones cols
        nc.vector.memset(ones_sb[:], 1.0)
        warm_ps = ptile([P, NTB], "warm_ps")
        nc.vector.memset(warm_ps[:], 1.0)
        warm_sb = ptile([P, NTB], "warm_sb", bf16)
        nc.scalar.activation(warm_sb[:], warm_ps[:], EXP, scale=0.125)
        ones_src = ones_sb[:].rearrange("p (b s) -> p b s", s=1)
        for va in (va1_sb, va2_sb):
            dst = va[:].rearrange("p (b s) -> p b s", s=65)[:, :, 64:65]
            nc.vector.tensor_copy(dst, ones_src)

        # ---- pools (PSUM: st 2*2 + yt 2*1 + ms 2*1 = 8 banks)
        with contextlib.ExitStack() as ctx:
            ps_st = ctx.enter_context(tc.tile_pool(name="ps_st", bufs=2, space="PSUM"))
            ps_yt = ctx.enter_context(tc.tile_pool(name="ps_yt", bufs=2, space="PSUM"))
            ps_ms = ctx.enter_context(tc.tile_pool(name="ps_ms", bufs=2, space="PSUM"))
            sb_e = ctx.enter_context(tc.tile_pool(name="sb_e", bufs=6))
            sb_vt = ctx.enter_context(tc.tile_pool(name="sb_vt", bufs=2))
            sb_sm = ctx.enter_context(tc.tile_pool(name="sb_sm", bufs=2))
            sb_ob = ctx.enter_context(tc.tile_pool(name="sb_ob", bufs=4))

            def proj_filler(c, w_sb, kind, vt_box):
                def f():
                    acc = ps_ms.tile([P, CH], f32, tag="ms", name=f"ps_{kind}{c}")
                    for k in range(KC):
                        nc.tensor.matmul(
                            acc[:],
                            w_sb[:, P * k : P * (k + 1)],
                            xT_sb[:, T * k + CH * c : T * k + CH * (c + 1)],
                            start=(k == 0),
                            stop=(k == KC - 1),
                        )
                    if kind == "k":
                        nc.vector.tensor_copy(kT_sb[:, CH * c : CH * (c + 1)], acc[:])
                    elif kind == "q":
                        nc.vector.tensor_copy(qT_sb[:, CH * c : CH * (c + 1)], acc[:])
                    else:
                        vt = sb_vt.tile([P, CH], bf16, tag="vt", name=f"vt{c}")
                        nc.vector.tensor_copy(vt[:], acc[:])
                        vt_box[0] = vt
                return f

            def trans_filler(c, b2, vt_box):
                def f():
                    jb = 4 * c + b2
                    tr = ps_ms.tile([P, CH], bf16, tag="ms", name=f"tr{jb}")
                    nc.tensor.transpose(
                        tr[:, 0:P], vt_box[0][:, P * b2 : P * (b2 + 1)], id_sb[:]
                    )
                    nc.vector.tensor_copy(va1_sb[:, 65 * jb : 65 * jb + 64], tr[:, 0:64])
                    nc.vector.tensor_copy(va2_sb[:, 65 * jb : 65 * jb + 64], tr[:, 64:P])
                return f

            def qkv_fillers(c, skip_qk=False):
                vt_box = [None]
                fs = []
                if not skip_qk:
                    fs += [proj_filler(c, wq_sb, "q", vt_box),
                           proj_filler(c, wk_sb, "k", vt_box)]
                fs.append(proj_filler(c, wv_sb, "v", vt_box))
                fs += [trans_filler(c, b2, vt_box) for b2 in range(4)]
                return fs

            def emit_st(c, j):
                stp = ps_st.tile([P, 2 * CH], f32, tag="st", name=f"st{c}_{j}")
                for h in (0, 1):
                    nc.tensor.matmul(
                        stp[:, CH * h : CH * (h + 1)],
                        kT_sb[64 * h : 64 * (h + 1), P * j : P * (j + 1)],
                        qT_sb[64 * h : 64 * (h + 1), CH * c : CH * (c + 1)],
                        start=True,
                        stop=True,
                    )
                et = sb_e.tile([P, 2 * CH], bf16, tag="e", name=f"e{c}_{j}")
                if j >= 4 * c:  # diagonal block
                    r = j - 4 * c
                    if r == 0:
                        nc.scalar.activation(et[:], stp[:], EXP, scale=0.125)
                    else:
                        for h in (0, 1):
                            nc.scalar.activation(
                                et[:, CH * h + P * r : CH * (h + 1)],
                                stp[:, CH * h + P * r : CH * (h + 1)],
                                EXP,
                                scale=0.125,
                            )
                        for h in (0, 1):
                            nc.vector.memset(et[:, CH * h : CH * h + P * r], 0.0)
                    # multiplicative 0/1 causal mask on the diagonal tile
                    for h in (0, 1):
                        o = CH * h + P * r
                        nc.vector.tensor_mul(et[:, o : o + P], et[:, o : o + P], mk_sb[:])
                else:
                    nc.scalar.activation(et[:], stp[:], EXP, scale=0.125)
                return et

            def emit_yt(c, j, et, yts):
                yt1, yt2 = yts
                njb = 4 * (c + 1)
                for h, yt, va in ((0, yt1, va1_sb), (1, yt2, va2_sb)):
                    nc.tensor.matmul(
                        yt[:],
                        va[:, 65 * j : 65 * j + 65],
                        et[:, CH * h : CH * (h + 1)],
                        start=(j == 0),
                        stop=(j == njb - 1),
                    )

            def emit_tail_a(c, yts):
                """Drain yt PSUM accumulators: denominators + unnormalized yT."""
                yt1, yt2 = yts
                spair = sb_sm.tile([65, CH], f32, tag="sp", name=f"sp{c}")
                nc.gpsimd.memset(spair[:], 1.0)
                nc.vector.tensor_copy(spair[0:1, :], yt1[64:65, :])
                nc.vector.tensor_copy(spair[64:65, :], yt2[64:65, :])
                nc.vector.tensor_copy(yT_sb[0:64, CH * c : CH * (c + 1)], yt1[0:64, :])
                nc.vector.tensor_copy(yT_sb[64:P, CH * c : CH * (c + 1)], yt2[0:64, :])
                return spair

            def tail_b_fillers(c, spair):
                """Reciprocal (split 4), normalize chunk c's yT, out-project."""
                fs = []
                rpair = sb_sm.tile([65, CH], f32r, tag="rp", name=f"rp{c}")

                def recip(qr):
                    def f():
                        with nc.allow_low_precision("f32r reciprocal for softmax"):
                            nc.vector.reciprocal(
                                rpair[:, P * qr : P * (qr + 1)],
                                spair[:, P * qr : P * (qr + 1)],
                            )
                    return f

                fs += [recip(qr) for qr in range(4)]

                def fnorm():
                    rb = ps_ms.tile([P, CH], f32, tag="ms", name=f"rb{c}")
                    # full-fp32 broadcast matmul (exact: sel is 0/1)
                    nc.tensor.matmul(rb[:], sel_sb[:], rpair[:], start=True, stop=True)
                    rbs = sb_sm.tile([P, CH], bf16, tag="rbs", name=f"rbs{c}")
                    nc.vector.tensor_copy(rbs[:], rb[:])
                    nc.gpsimd.tensor_mul(
                        yT_sb[:, CH * c : CH * (c + 1)],
                        yT_sb[:, CH * c : CH * (c + 1)],
                        rbs[:],
                    )
                fs.append(fnorm)

                def outp(b2):
                    def f():
                        tb = 4 * c + b2
                        op = ps_ms.tile([P, CH], f32, tag="ms", name=f"op{tb}")
                        nc.tensor.matmul(
                            op[:],
                            yT_sb[:, P * tb : P * (tb + 1)],
                            wo_sb[:],
                            start=True,
                            stop=True,
                        )
                        ob = sb_ob.tile([P, CH], f32, tag="ob", name=f"ob{tb}")
                        nc.vector.tensor_copy(ob[:], op[:])
                        nc.sync.dma_start(out_d[P * tb : P * (tb + 1), :], ob[:])
                    return f

                fs += [outp(b2) for b2 in range(4)]
                return fs

            # ---- main schedule: one continuous pipeline over all (c, j)
            # chunk 0's q/k projections run inline so scores can start ASAP
            vt0_box = [None]
            proj_filler(0, wq_sb, "q", vt0_box)()
            proj_filler(0, wk_sb, "k", vt0_box)()

            fq_hi = qkv_fillers(0, skip_qk=True)  # v+transposes of chunk 0
            fq_lo = []
            hi_total, hi_popped = len(fq_hi), 0
            pending = []   # (c, j, et) awaiting av emission
            yts_cur = None
            tails = {}     # c -> yts awaiting tail_a

            jobs = [(c, j) for c in range(NCH) for j in range(4 * (c + 1))]
            for c, j in jobs:
                if j == 0:
                    # pace chunk c's own remaining hi fillers + next chunk's qkv
                    if c + 1 < NCH:
                        fq_hi += qkv_fillers(c + 1)
                    hi_total, hi_popped = len(fq_hi), 0
                    njb = 4 * (c + 1)
                et = emit_st(c, j)
                pending.append((c, j, et))
                # paced high-priority fillers (QKV for the next chunk)
                want = (hi_total * (j + 1) + njb - 1) // njb
                popped_this_j = False
                while hi_popped < want and fq_hi:
                    fq_hi.pop(0)()
                    hi_popped += 1
                    popped_this_j = True
                if not popped_this_j and fq_lo:
                    fq_lo.pop(0)()
                if len(pending) > 2:
                    pc, pj, pet = pending.pop(0)
                    if pj == 0:
                        yts_cur = (
                            ps_yt.tile([65, CH], f32, tag="yt", name=f"yt1_{pc}"),
                            ps_yt.tile([65, CH], f32, tag="yt", name=f"yt2_{pc}"),
                        )
                        tails[pc] = yts_cur
                    emit_yt(pc, pj, pet, tails[pc])
                    if pj == 4 * (pc + 1) - 1:  # chunk pc fully accumulated
                        while fq_lo:  # drain stale tail work (deadlock safety)
                            fq_lo.pop(0)()
                        spair = emit_tail_a(pc, tails.pop(pc))
                        fq_lo = tail_b_fillers(pc, spair)
            # drain remaining avs and tails
            for pc, pj, pet in pending:
                if pj == 0:
                    yts_cur = (
                        ps_yt.tile([65, CH], f32, tag="yt", name=f"yt1_{pc}"),
                        ps_yt.tile([65, CH], f32, tag="yt", name=f"yt2_{pc}"),
                    )
                    tails[pc] = yts_cur
                emit_yt(pc, pj, pet, tails[pc])
                if pj == 4 * (pc + 1) - 1:
                    while fq_lo:
                        fq_lo.pop(0)()
                    spair = emit_tail_a(pc, tails.pop(pc))
                    fq_lo = tail_b_fillers(pc, spair)
            while fq_lo:
                fq_lo.pop(0)()

    nc.compile()
    return nc


def _host_inputs(x, Wq, Wk, Wv, Wout):
    """Per-core input maps. Core c: batch b=c//4, head-group g=c%4."""
    import ml_dtypes

    bf16 = ml_dtypes.bfloat16
    x = np.asarray(x, dtype=np.float32)
    Wq = np.asarray(Wq, dtype=np.float32)
    Wk = np.asarray(Wk, dtype=np.float32)
    Wv = np.asarray(Wv, dtype=np.float32)
    Wout = np.asarray(Wout, dtype=np.float32)

    # multiplicative causal mask for the 128x128 diagonal tile:
    # ST[t2 row, t1 col] valid iff col >= row
    col = np.arange(P)[None, :]
    row = np.arange(P)[:, None]
    mk = (col >= row).astype(bf16)
    sel = np.zeros((65, P), dtype=np.float32)
    sel[0, 0:64] = 1.0
    sel[64, 64:P] = 1.0
    idm = np.eye(P, dtype=bf16)

    def arrange_w(Wc):  # Wc: [128 feat, 512 c] -> lhsT layout [p, (k m)]
        return np.concatenate(
            [np.ascontiguousarray(Wc[:, P * k : P * (k + 1)].T) for k in range(KC)],
            axis=1,
        ).astype(bf16)

    in_maps = []
    for core in range(8):
        b, g = core // 4, core % 4
        rows = slice(P * g, P * (g + 1))
        in_maps.append(
            {
                "xT": np.ascontiguousarray(x[b].T).astype(bf16),
                "wq": arrange_w(Wq[rows]),
                "wk": arrange_w(Wk[rows]),
                "wv": arrange_w(Wv[rows]),
                "wo": np.ascontiguousarray(Wout[:, rows].T).astype(bf16),
                "mk": mk,
                "sel": sel,
                "idm": idm,
            }
        )
    return in_maps


def _get_compiled():
    global _COMPILED
    if _COMPILED is None:
        _COMPILED = _build()
    return _COMPILED


def run_on_hw(x, Wq, Wk, Wv, Wout, trace=False):
    """Returns (full_output [B,T,C], exec_time_ns_or_None)."""
    _import_concourse()
    from concourse import bass_utils

    nc = _get_compiled()
    in_maps = _host_inputs(x, Wq, Wk, Wv, Wout)
    res = bass_utils.run_bass_kernel_spmd(
        nc, in_maps, list(range(8)), trace=trace
    )
    global LAST_RESULT
    LAST_RESULT = res
    parts = [res.results[i]["out"] for i in range(8)]
    out = np.stack(
        [
            parts[0] + parts[1] + parts[2] + parts[3],
            parts[4] + parts[5] + parts[6] + parts[7],
        ]
    ).astype(np.float32)
    return out, res.exec_time_ns


def kernel(x, Wq, Wk, Wv, Wout):
    out, _ = run_on_hw(x, Wq, Wk, Wv, Wout, trace=False)
    return out


if __name__ == "__main__":
    # smoke test with random data (no reference)
    rng = np.random.default_rng(0)
    x = rng.standard_normal((B, T, C), dtype=np.float32)
    s = 1.0 / np.sqrt(C)
    ws = [rng.standard_normal((C, C), dtype=np.float32) * s for _ in range(4)]
    out = kernel(x, *ws)
    print("out", out.shape, out.dtype, np.abs(out).mean())


# revision 11
# speedup vs baseline: 1.1870x; 1.1870x over previous
"""Causal self-attention Trainium2 Bass kernel (v4: continuous pipeline).

Problem: B=2, T=4096, C=512, H=8 heads, D=64.
  q = x@Wq.T, k = x@Wk.T, v = x@Wv.T  (per-head split)
  att = softmax(causal(q k^T / sqrt(D)));  y = att @ v;  out = y @ Wout.T

Sharding: 8 cores = 2 batches x 4 head-groups (2 heads each).
Each core computes, for its batch b and heads {2g, 2g+1}:
  - feature-major qT,kT [128, T] bf16 via PE matmuls (bf16, 1 cyc/row)
  - one continuous software pipeline over all (chunk c, t2-block j) pairs,
    crossing chunk boundaries without draining: transposed scores
    ST[t2, t1] = kT^T qT (two 64-partition row-groups run concurrently on
    the PE), exp on ACT (scale=1/8) over the causally-valid column range,
    diagonal 128x128 tiles masked multiplicatively (0/1 bf16, DVE) after
    exp, then yT_aug[65, t1] accumulation (ones column -> denominators).
  - QKV projections for the next chunk and the previous chunk's
    normalization/out-projection are interleaved as paced PE "fillers" so
    the ACT (exp) engine -- the bottleneck -- never starves; the
    reciprocal is split into 4 quarters to keep the in-order DVE queue
    responsive.
Host sums the 4 partial outputs per batch (row-parallel out projection).
"""

import os
import sys

import numpy as np

B, T, C = 2, 4096, 512
H, D = 8, 64
P = 128          # partitions / t2-block size
CH = 512         # t1 chunk width
NCH = T // CH    # 8 chunks
NTB = T // P     # 32 t-blocks
KC = C // P      # 4 contraction chunks for projections

_COMPILED = None


def _import_concourse():
    try:
        import concourse.bass  # noqa: F401
    except ImportError:
        for p in ("/opt/trn_rl_repo", os.path.expanduser("~/.axon_site/_ro/trn_rl_repo")):
            if os.path.isdir(p) and p not in sys.path:
                sys.path.insert(0, p)
        import concourse.bass  # noqa: F401


def _build():
    """Build + compile the SPMD Bass program (same program on all 8 cores)."""
    _import_concourse()
    import concourse.bass as bass  # noqa: F401
    import concourse.tile as tile
    from concourse import bacc, mybir

    f32 = mybir.dt.float32
    f32r = mybir.dt.float32r
    bf16 = mybir.dt.bfloat16
    EXP = mybir.ActivationFunctionType.Exp

    nc = bacc.Bacc("TRN2", target_bir_lowering=False, debug=False, num_devices=8)

    xT_d = nc.dram_tensor("xT", [C, T], bf16, kind="ExternalInput").ap()
    wq_d = nc.dram_tensor("wq", [P, C], bf16, kind="ExternalInput").ap()
    wk_d = nc.dram_tensor("wk", [P, C], bf16, kind="ExternalInput").ap()
    wv_d = nc.dram_tensor("wv", [P, C], bf16, kind="ExternalInput").ap()
    wo_d = nc.dram_tensor("wo", [P, C], bf16, kind="ExternalInput").ap()
    mk_d = nc.dram_tensor("mk", [P, P], bf16, kind="ExternalInput").ap()
    sel_d = nc.dram_tensor("sel", [65, P], f32, kind="ExternalInput").ap()
    id_d = nc.dram_tensor("idm", [P, P], bf16, kind="ExternalInput").ap()
    out_d = nc.dram_tensor("out", [T, C], bf16, kind="ExternalOutput").ap()

    import contextlib

    with tile.TileContext(nc) as tc, contextlib.ExitStack() as _pctx:
        # ---- persistent SBUF tensors
        persist = _pctx.enter_context(tc.tile_pool(name="persist", bufs=1))

        def ptile(shape, name, dt=f32):
            return persist.tile(shape, dt, name=name, tag=name)

        xT_sb = ptile([P, KC * T], "xT_sb", bf16)      # 4 MB
        wq_sb = ptile([P, C], "wq_sb", bf16)
        wk_sb = ptile([P, C], "wk_sb", bf16)
        wv_sb = ptile([P, C], "wv_sb", bf16)
        wo_sb = ptile([P, C], "wo_sb", bf16)
        mk_sb = ptile([P, P], "mk_sb", bf16)           # 0/1 lower-valid mask
        sel_sb = ptile([65, P], "sel_sb", f32)
        id_sb = ptile([P, P], "id_sb", bf16)
        qT_sb = ptile([P, T], "qT_sb", bf16)
        kT_sb = ptile([P, T], "kT_sb", bf16)
        va1_sb = ptile([P, NTB * 65], "va1_sb", bf16)
        va2_sb = ptile([P, NTB * 65], "va2_sb", bf16)
        yT_sb = ptile([P, T], "yT_sb", bf16)

        # ---- input DMAs: q/k weights + first xT quarter first so chunk-0
        # QKV starts ASAP; the rest follows.
        TQ = T // 4

        def dma_xq(q):
            for k in range(KC):
                nc.sync.dma_start(
                    xT_sb[:, T * k + TQ * q : T * k + TQ * (q + 1)],
                    xT_d[P * k : P * (k + 1), TQ * q : TQ * (q + 1)],
                )

        nc.sync.dma_start(wq_sb[:], wq_d[:])
        nc.sync.dma_start(wk_sb[:], wk_d[:])
        dma_xq(0)
        nc.sync.dma_start(wv_sb[:], wv_d[:])
        nc.sync.dma_start(wo_sb[:], wo_d[:])
        nc.sync.dma_start(sel_sb[:], sel_d[:])
        nc.sync.dma_start(id_sb[:], id_d[:])
        nc.sync.dma_start(mk_sb[:], mk_d[:])
        for q in range(1, 4):
            dma_xq(q)
        ones_sb = ptile([P, NTB], "ones_sb", bf16)  # staging for vaug ones cols
        nc.vector.memset(ones_sb[:], 1.0)
        warm_ps = ptile([P, NTB], "warm_ps")
        nc.vector.memset(warm_ps[:], 1.0)
        warm_sb = ptile([P, NTB], "warm_sb", bf16)
        nc.scalar.activation(warm_sb[:], warm_ps[:], EXP, scale=0.125)
        ones_src = ones_sb[:].rearrange("p (b s) -> p b s", s=1)
        for va in (va1_sb, va2_sb):
            dst = va[:].rearrange("p (b s) -> p b s", s=65)[:, :, 64:65]
            nc.vector.tensor_copy(dst, ones_src)

        # ---- pools (PSUM: st 2*2 + yt 2*1 + ms 2*1 = 8 banks)
        with contextlib.ExitStack() as ctx:
            ps_st = ctx.enter_context(tc.tile_pool(name="ps_st", bufs=2, space="PSUM"))
            ps_yt = ctx.enter_context(tc.tile_pool(name="ps_yt", bufs=2, space="PSUM"))
            ps_ms = ctx.enter_context(tc.tile_pool(name="ps_ms", bufs=2, space="PSUM"))
            sb_e = ctx.enter_context(tc.tile_pool(name="sb_e", bufs=6))
            sb_vt = ctx.enter_context(tc.tile_pool(name="sb_vt", bufs=2))
            sb_sm = ctx.enter_context(tc.tile_pool(name="sb_sm", bufs=2))
            sb_ob = ctx.enter_context(tc.tile_pool(name="sb_ob", bufs=4))

            def proj_filler(c, w_sb, kind, vt_box):
                def f():
                    acc = ps_ms.tile([P, CH], f32, tag="ms", name=f"ps_{kind}{c}")
                    for k in range(KC):
                        nc.tensor.matmul(
                            acc[:],
                            w_sb[:, P * k : P * (k + 1)],
                            xT_sb[:, T * k + CH * c : T * k + CH * (c + 1)],
                            start=(k == 0),
                            stop=(k == KC - 1),
                        )
                    if kind == "k":
                        nc.vector.tensor_copy(kT_sb[:, CH * c : CH * (c + 1)], acc[:])
                    elif kind == "q":
                        nc.vector.tensor_copy(qT_sb[:, CH * c : CH * (c + 1)], acc[:])
                    else:
                        vt = sb_vt.tile([P, CH], bf16, tag="vt", name=f"vt{c}")
                        nc.vector.tensor_copy(vt[:], acc[:])
                        vt_box[0] = vt
                return f

            def trans_filler(c, b2, vt_box):
                def f():
                    jb = 4 * c + b2
                    tr = ps_ms.tile([P, CH], bf16, tag="ms", name=f"tr{jb}")
                    nc.tensor.transpose(
                        tr[:, 0:P], vt_box[0][:, P * b2 : P * (b2 + 1)], id_sb[:]
                    )
                    nc.vector.tensor_copy(va1_sb[:, 65 * jb : 65 * jb + 64], tr[:, 0:64])
                    nc.vector.tensor_copy(va2_sb[:, 65 * jb : 65 * jb + 64], tr[:, 64:P])
                return f

            def qkv_fillers(c, skip_qk=False):
                vt_box = [None]
                fs = []
                if not skip_qk:
                    fs += [proj_filler(c, wq_sb, "q", vt_box),
                           proj_filler(c, wk_sb, "k", vt_box)]
                fs.append(proj_filler(c, wv_sb, "v", vt_box))
                fs += [trans_filler(c, b2, vt_box) for b2 in range(4)]
                return fs

            def emit_st(c, j):
                """Scores + exp for block (c, j). Diagonal blocks are trimmed
                to the causally valid column range [P*r:) — columns left of it
                are never computed, exp'd, nor read by the trimmed av."""
                r = j - 4 * c  # >= 0 on the diagonal strip
                o = P * r if r > 0 else 0
                stp = ps_st.tile([P, 2 * CH], f32, tag="st", name=f"st{c}_{j}")
                for h in (0, 1):
                    nc.tensor.matmul(
                        stp[:, CH * h + o : CH * (h + 1)],
                        kT_sb[64 * h : 64 * (h + 1), P * j : P * (j + 1)],
                        qT_sb[64 * h : 64 * (h + 1), CH * c + o : CH * (c + 1)],
                        start=True,
                        stop=True,
                    )
                et = sb_e.tile([P, 2 * CH], bf16, tag="e", name=f"e{c}_{j}")
                if r <= 0:
                    nc.scalar.activation(et[:], stp[:], EXP, scale=0.125)
                else:
                    for h in (0, 1):
                        nc.scalar.activation(
                            et[:, CH * h + o : CH * (h + 1)],
                            stp[:, CH * h + o : CH * (h + 1)],
                            EXP,
                            scale=0.125,
                        )
                if r >= 0:
                    # multiplicative 0/1 causal mask on the diagonal tile
                    for h in (0, 1):
                        d = CH * h + o
                        nc.vector.tensor_mul(et[:, d : d + P], et[:, d : d + P], mk_sb[:])
                return et

            def emit_yt(c, j, et, yts):
                yt1, yt2 = yts
                njb = 4 * (c + 1)
                r = j - 4 * c
                o = P * r if r > 0 else 0
                for h, yt, va in ((0, yt1, va1_sb), (1, yt2, va2_sb)):
                    nc.tensor.matmul(
                        yt[:, o:CH],
                        va[:, 65 * j : 65 * j + 65],
                        et[:, CH * h + o : CH * (h + 1)],
                        start=(j == 0),
                        stop=(j == njb - 1),
                    )

            def emit_tail_a(c, yts):
                """Drain yt PSUM accumulators: denominators + unnormalized yT."""
                yt1, yt2 = yts
                spair = sb_sm.tile([65, CH], f32, tag="sp", name=f"sp{c}")
                nc.gpsimd.memset(spair[:], 1.0)
                nc.vector.tensor_copy(spair[0:1, :], yt1[64:65, :])
                nc.vector.tensor_copy(spair[64:65, :], yt2[64:65, :])
                nc.vector.tensor_copy(yT_sb[0:64, CH * c : CH * (c + 1)], yt1[0:64, :])
                nc.vector.tensor_copy(yT_sb[64:P, CH * c : CH * (c + 1)], yt2[0:64, :])
                return spair

            def tail_b_fillers(c, spair):
                """Reciprocal, normalize chunk c's yT, out-project."""
                fs = []
                rpair = sb_sm.tile([65, CH], f32, tag="rp", name=f"rp{c}")

                def recip():
                    # ~51-ULP single-instruction approx; denominators are in
                    # [1, ~7000] so no edge cases
                    nc.vector.reciprocal_approx_fast(rpair[:], spair[:])

                fs.append(recip)

                def fnorm():
                    rb = ps_ms.tile([P, CH], f32, tag="ms", name=f"rb{c}")
                    # full-fp32 broadcast matmul (exact: sel is 0/1)
                    nc.tensor.matmul(rb[:], sel_sb[:], rpair[:], start=True, stop=True)
                    rbs = sb_sm.tile([P, CH], bf16, tag="rbs", name=f"rbs{c}")
                    nc.vector.tensor_copy(rbs[:], rb[:])
                    nc.gpsimd.tensor_mul(
                        yT_sb[:, CH * c : CH * (c + 1)],
                        yT_sb[:, CH * c : CH * (c + 1)],
                        rbs[:],
                    )
                fs.append(fnorm)

                def outp(b2):
                    def f():
                        tb = 4 * c + b2
                        op = ps_ms.tile([P, CH], f32, tag="ms", name=f"op{tb}")
                        nc.tensor.matmul(
                            op[:],
                            yT_sb[:, P * tb : P * (tb + 1)],
                            wo_sb[:],
                            start=True,
                            stop=True,
                        )
                        ob = sb_ob.tile([P, CH], bf16, tag="ob", name=f"ob{tb}")
                        nc.vector.tensor_copy(ob[:], op[:])
                        nc.sync.dma_start(out_d[P * tb : P * (tb + 1), :], ob[:])
                    return f

                fs += [outp(b2) for b2 in range(4)]
                return fs

            # ---- main schedule: one continuous pipeline over all (c, j)
            # chunk 0's q/k projections run inline so scores can start ASAP
            vt0_box = [None]
            proj_filler(0, wq_sb, "q", vt0_box)()
            proj_filler(0, wk_sb, "k", vt0_box)()

            fq_hi = qkv_fillers(0, skip_qk=True)  # v+transposes of chunk 0
            fq_lo = []
            hi_total, hi_popped = len(fq_hi), 0
            pending = []   # (c, j, et) awaiting av emission
            yts_cur = None
            tails = {}     # c -> yts awaiting tail_a

            jobs = [(c, j) for c in range(NCH) for j in range(4 * (c + 1))]
            for c, j in jobs:
                if j == 0:
                    # pace chunk c's own remaining hi fillers + next chunk's qkv
                    if c + 1 < NCH:
                        fq_hi += qkv_fillers(c + 1)
                    hi_total, hi_popped = len(fq_hi), 0
                    njb = 4 * (c + 1)
                et = emit_st(c, j)
                pending.append((c, j, et))
                # paced high-priority fillers (QKV for the next chunk)
                want = (hi_total * (j + 1) + njb - 1) // njb
                popped_this_j = False
                while hi_popped < want and fq_hi:
                    fq_hi.pop(0)()
                    hi_popped += 1
                    popped_this_j = True
                if not popped_this_j and fq_lo:
                    fq_lo.pop(0)()
                if len(pending) > 2:
                    pc, pj, pet = pending.pop(0)
                    if pj == 0:
                        yts_cur = (
                            ps_yt.tile([65, CH], f32, tag="yt", name=f"yt1_{pc}"),
                            ps_yt.tile([65, CH], f32, tag="yt", name=f"yt2_{pc}"),
                        )
                        tails[pc] = yts_cur
                    emit_yt(pc, pj, pet, tails[pc])
                    if pj == 4 * (pc + 1) - 1:  # chunk pc fully accumulated
                        while fq_lo:  # drain stale tail work (deadlock safety)
                            fq_lo.pop(0)()
                        spair = emit_tail_a(pc, tails.pop(pc))
                        fq_lo = tail_b_fillers(pc, spair)
            # drain remaining avs and tails
            for pc, pj, pet in pending:
                if pj == 0:
                    yts_cur = (
                        ps_yt.tile([65, CH], f32, tag="yt", name=f"yt1_{pc}"),
                        ps_yt.tile([65, CH], f32, tag="yt", name=f"yt2_{pc}"),
                    )
                    tails[pc] = yts_cur
                emit_yt(pc, pj, pet, tails[pc])
                if pj == 4 * (pc + 1) - 1:
                    while fq_lo:
                        fq_lo.pop(0)()
                    spair = emit_tail_a(pc, tails.pop(pc))
                    fq_lo = tail_b_fillers(pc, spair)
            while fq_lo:
                fq_lo.pop(0)()

    nc.compile()
    return nc


def _host_inputs(x, Wq, Wk, Wv, Wout):
    """Per-core input maps. Core c: batch b=c//4, head-group g=c%4."""
    import ml_dtypes

    bf16 = ml_dtypes.bfloat16
    x = np.asarray(x, dtype=np.float32)
    Wq = np.asarray(Wq, dtype=np.float32)
    Wk = np.asarray(Wk, dtype=np.float32)
    Wv = np.asarray(Wv, dtype=np.float32)
    Wout = np.asarray(Wout, dtype=np.float32)

    # multiplicative causal mask for the 128x128 diagonal tile:
    # ST[t2 row, t1 col] valid iff col >= row
    col = np.arange(P)[None, :]
    row = np.arange(P)[:, None]
    mk = (col >= row).astype(bf16)
    sel = np.zeros((65, P), dtype=np.float32)
    sel[0, 0:64] = 1.0
    sel[64, 64:P] = 1.0
    idm = np.eye(P, dtype=bf16)

    def arrange_w(Wc):  # Wc: [128 feat, 512 c] -> lhsT layout [p, (k m)]
        return np.concatenate(
            [np.ascontiguousarray(Wc[:, P * k : P * (k + 1)].T) for k in range(KC)],
            axis=1,
        ).astype(bf16)

    in_maps = []
    for core in range(8):
        b, g = core // 4, core % 4
        rows = slice(P * g, P * (g + 1))
        in_maps.append(
            {
                "xT": np.ascontiguousarray(x[b].T).astype(bf16),
                "wq": arrange_w(Wq[rows]),
                "wk": arrange_w(Wk[rows]),
                "wv": arrange_w(Wv[rows]),
                "wo": np.ascontiguousarray(Wout[:, rows].T).astype(bf16),
                "mk": mk,
                "sel": sel,
                "idm": idm,
            }
        )
    return in_maps


def _get_compiled():
    global _COMPILED
    if _COMPILED is None:
        _COMPILED = _build()
    return _COMPILED


def run_on_hw(x, Wq, Wk, Wv, Wout, trace=False):
    """Returns (full_output [B,T,C], exec_time_ns_or_None)."""
    _import_concourse()
    from concourse import bass_utils

    nc = _get_compiled()
    in_maps = _host_inputs(x, Wq, Wk, Wv, Wout)
    res = bass_utils.run_bass_kernel_spmd(
        nc, in_maps, list(range(8)), trace=trace
    )
    global LAST_RESULT
    LAST_RESULT = res
    parts = [np.asarray(res.results[i]["out"], dtype=np.float32) for i in range(8)]
    out = np.stack(
        [
            parts[0] + parts[1] + parts[2] + parts[3],
            parts[4] + parts[5] + parts[6] + parts[7],
        ]
    ).astype(np.float32)
    return out, res.exec_time_ns


def kernel(x, Wq, Wk, Wv, Wout):
    out, _ = run_on_hw(x, Wq, Wk, Wv, Wout, trace=False)
    return out


if __name__ == "__main__":
    # smoke test with random data (no reference)
    rng = np.random.default_rng(0)
    x = rng.standard_normal((B, T, C), dtype=np.float32)
    s = 1.0 / np.sqrt(C)
    ws = [rng.standard_normal((C, C), dtype=np.float32) * s for _ in range(4)]
    out = kernel(x, *ws)
    print("out", out.shape, out.dtype, np.abs(out).mean())
